# revision 1
# baseline (speedup 1.0000x reference)
"""Trainium2 Bass kernel for nn_DSNet (dense_cnn).

Math: the reference computes
  ref  = conv1d(refer, w_seq, b_seq)            # (1, 512, 32768), k=3 over time
  seq  = concat([ref, x.T], time) -> (65536, 512)
  splits = seq.reshape(32768, 2, 512)
  s    = relu(conv1d(splits, w1, b1))[:, 0, :]  # k=3 over the 512 axis
  s    = relu(s @ w2[:,:,1].T + b2)
  out  = sigmoid(s @ w3[:,:,1].T + b3)          # (32768, 64, 1)

Key folding: for the first 16384 splits (the `ref` half), linear_seq + concat +
split + conv1 collapse into ONE stride-2 4-tap conv applied directly to
`refer` with host-precomputed effective weights Weff[d, i, tau] / beff[d]:
  s1[n, d] = relu( sum_{i, tau} refer[i, 2n + tau - 1] * Weff[d,i,tau] + beff[d] )
For the x half, conv1 along the 512-axis becomes banded matmuls on xT windows.
Everything stays in (channel, split) layout on-chip so no transposes are needed;
matmul2/matmul3 contract over the channel partition dim.

dtypes: conv inputs (refer, x windows, conv weights) are bf16 (halves HBM
traffic; rel err ~1e-3 at the sigmoid output); the psum accumulation and the
s1/h/mm2/mm3 chain stay fp32, with matmuls run as float32r (full-rate PE).

DMA layout: weights are packed into two bundle tensors (bf16 + fp32) and x
windows are stored chunk-major so each n-chunk loads with a single dma_start
(~23 descriptors total; HWDGE costs ~625ns per dma_start, serialized). A few
dummy matmuls + activations at the top warm the PE clock and the ACT tables
while the first DMAs are in flight.

Sharding: splits are sharded 8 ways; core c handles ref-part splits
[2048c, 2048(c+1)) and x-part splits 16384 + [2048c, 2048(c+1)).
"""
import sys

import numpy as np

sys.path.insert(0, "/opt/trn_rl_repo")

D_IN, D_SEQ, D_H, D_OUT = 64, 512, 128, 64
T_REF = 32768
N_CORES = 8
NCHUNK = 512  # splits per n-chunk
WIN = [(128, 126), (128, 126), (128, 126), (128, 126), (10, 8)]  # (K, M) per x window
WB16_COLS = 2 * 4 * 128 + 2 * 126 + 2 * 8 + 512 + 640 + 64  # = 2508
WF32_COLS = 4 + 3  # beff | b2,b3,b1 = 7

_CACHE = {}


def _build_nc():
    import concourse.bacc as bacc
    import concourse.bass as bass
    import concourse.mybir as mybir
    import concourse.tile as tile

    f32 = mybir.dt.float32
    bf16 = mybir.dt.bfloat16
    AF = mybir.ActivationFunctionType
    ALU = mybir.AluOpType
    f32r = mybir.dt.float32r

    nc = bacc.Bacc("TRN2", target_bir_lowering=False, debug=False, num_devices=N_CORES)

    refer_sl = nc.dram_tensor("refer_sl", [D_IN, 4100], bf16, kind="ExternalInput").ap()
    xwc_d = nc.dram_tensor("xwc", [4, 128, 4, 1024], bf16, kind="ExternalInput").ap()
    xw4_d = nc.dram_tensor("xw4", [10, 4096], bf16, kind="ExternalInput").ap()
    wb16_d = nc.dram_tensor("wb16", [128, WB16_COLS], bf16, kind="ExternalInput").ap()
    wf32_d = nc.dram_tensor("wf32", [128, WF32_COLS], f32, kind="ExternalInput").ap()
    out_d = nc.dram_tensor("res", [D_OUT, 4096], f32, kind="ExternalOutput").ap()

    with tile.TileContext(nc) as tc:
        with (
            tc.tile_pool(name="wp", bufs=1) as wp,
            tc.tile_pool(name="dp", bufs=2) as dp,
            tc.tile_pool(name="op", bufs=2) as op,
            tc.tile_pool(name="s1p", bufs=10) as s1p,
            tc.tile_pool(name="hp", bufs=2) as hp,
            tc.tile_pool(name="ppc", bufs=5, space=bass.MemorySpace.PSUM) as ppc,
            tc.tile_pool(name="pph", bufs=2, space=bass.MemorySpace.PSUM) as pph,
            tc.tile_pool(name="ppo", bufs=1, space=bass.MemorySpace.PSUM) as ppo,
        ):
            # PE warmup: dummy matmuls on a memset tile keep the PE busy
            # during the initial DMA wait so real matmuls start at full clock
            warm = wp.tile([1, NCHUNK], bf16)
            nc.gpsimd.memset(warm[:], 0.0)
            wact = wp.tile([1, 16], f32)
            nc.scalar.activation(wact[:], warm[0:1, 0:16], AF.Relu)
            nc.scalar.activation(wact[:], warm[0:1, 0:16], AF.Sigmoid)
            psw = ppo.tile([1, NCHUNK], f32, tag="o", name="psw")
            for _ in range(6):
                nc.tensor.matmul(
                    psw[0:1, :], warm[0:1, 0:1], warm[0:1, :],
                    start=True, stop=True,
                )

            wb16 = wp.tile([128, WB16_COLS], bf16)
            nc.sync.dma_start(wb16[:, 0:1292], wb16_d[:, 0:1292])

            # refer with two tau-shifted copies stacked in the partition dim;
            # loaded in per-chunk column slices interleaved with x chunk loads
            refer2 = wp.tile([128, 4100], bf16)

            def load_ref(b):
                c0, c1 = 1024 * b, 1024 * b + 1026
                nc.sync.dma_start(refer2[0:64, c0:c1], refer_sl[:, c0:c1])
                nc.sync.dma_start(refer2[64:128, c0:c1], refer_sl[:, c0 + 2:c1 + 2])

            load_ref(0)
            wf32 = wp.tile([128, WF32_COLS], f32)
            nc.sync.dma_start(wf32[:], wf32_d[:])
            nc.sync.dma_start(wb16[:, 1292:2508], wb16_d[:, 1292:2508])
            xwin4 = wp.tile([10, 4096], bf16)
            nc.sync.dma_start(xwin4[:], xw4_d[:])

            xt_tiles = {}

            def load_xt(b):
                xt = dp.tile([128, 4, 1024], bf16, tag="xt", name=f"xt_{b}")
                nc.sync.dma_start(xt[:], xwc_d[b])
                xt_tiles[b] = xt

            load_xt(0)
            for b in range(1, 4):
                load_ref(b)
                load_xt(b)

            # weight slice views
            def wefft(t0, q):
                c = t0 * 512 + q * 128
                return wb16[:, c:c + 128]

            def bandA(cp):
                c = 1024 + cp * 126
                return wb16[:, c:c + 126]

            def band4(cp):
                c = 1276 + cp * 8
                return wb16[0:10, c:c + 8]

            def w2r(q):
                return wb16[:, 1292 + q * 128:1292 + (q + 1) * 128]

            def w2x(j):
                return wb16[:, 1804 + j * 128:1804 + (j + 1) * 128]

            w3m = wb16[:, 2444:2508]
            beff = wf32[:, 0:4]
            b2v = wf32[:, 4:5]
            b3v = wf32[0:64, 5:6]
            b1v = wf32[:, 6:7]

            def tail(s1_tiles, sizes, w2_sel, col0, b):
                """mm2 (contract 512) -> relu -> mm3 (128->64) -> sigmoid."""
                hps = pph.tile([128, NCHUNK], f32, tag="h", name=f"hps_{col0}_{b}")
                nct = len(sizes)
                for j in range(nct):
                    m = sizes[j]
                    nc.tensor.matmul(
                        hps[:],
                        w2_sel(j)[0:m, :],
                        s1_tiles[j][0:m, :],
                        start=(j == 0),
                        stop=(j == nct - 1),
                    )
                hsb = hp.tile([128, NCHUNK], bf16, tag="hs", name=f"hsb_{col0}_{b}")
                ops = ppo.tile([64, NCHUNK], f32, tag="o", name=f"ops_{col0}_{b}")
                osb = op.tile([64, NCHUNK], f32, tag="os", name=f"osb_{col0}_{b}")
                c0 = col0 + NCHUNK * b
                nc.vector.tensor_scalar(
                    hsb[:], hps[:], b2v, 0.0, ALU.add, ALU.max
                )
                nc.tensor.matmul(ops[:], w3m, hsb[:], start=True, stop=True)
                nc.scalar.activation(osb[:], ops[:], AF.Sigmoid, bias=b3v)
                nc.sync.dma_start(out_d[:, c0:c0 + NCHUNK], osb[:])

            def ref_chunk(b):
                s1_tiles = []
                for q in range(4):
                    ps = ppc.tile([128, NCHUNK], f32, tag="cv", name=f"psr_{b}_{q}")
                    for t0 in (0, 1):
                        rhs = refer2[:, 1024 * b + t0: 1024 * b + t0 + 2 * NCHUNK: 2]
                        nc.tensor.matmul(
                            ps[:], wefft(t0, q), rhs,
                            start=(t0 == 0), stop=(t0 == 1),
                        )
                    s1 = s1p.tile([128, NCHUNK], bf16, tag="s1", name=f"s1r_{b}_{q}")
                    if q % 2 == 0:
                        nc.scalar.activation(s1[:], ps[:], AF.Relu, bias=beff[:, q:q + 1])
                    else:
                        nc.vector.tensor_scalar(
                            s1[:], ps[:], beff[:, q:q + 1], 0.0, ALU.add, ALU.max
                        )
                    s1_tiles.append(s1)
                tail(s1_tiles, [128, 128, 128, 128], w2r, 0, b)

            def x_chunk(b):
                s1_tiles = []
                xt = xt_tiles[b]
                for j, (K, M) in enumerate(WIN):
                    ps = ppc.tile([128, NCHUNK], f32, tag="cv", name=f"psx_{b}_{j}")
                    for cp in (0, 1):
                        if j < 4:
                            rhs = xt[:, j, cp: 1024: 2]
                            lhsT = bandA(cp)
                        else:
                            rhs = xwin4[:, 1024 * b + cp: 1024 * (b + 1): 2]
                            lhsT = band4(cp)
                        nc.tensor.matmul(
                            ps[0:M, :], lhsT, rhs,
                            start=(cp == 0), stop=(cp == 1),
                        )
                    s1 = s1p.tile([128, NCHUNK], bf16, tag="s1", name=f"s1x_{b}_{j}")
                    if j % 2 == 0:
                        nc.scalar.activation(
                            s1[0:M, :], ps[0:M, :], AF.Relu, bias=b1v[0:M, :]
                        )
                    else:
                        nc.vector.tensor_scalar(
                            s1[0:M, :], ps[0:M, :], b1v[0:M, :], 0.0, ALU.add, ALU.max
                        )
                    s1_tiles.append(s1)
                tail(s1_tiles, [m for _, m in WIN], w2x, 2048, b)

            for b in range(4):
                ref_chunk(b)
                x_chunk(b)

    nc.compile()
    return nc


def _host_prep_weights(w_seq, b_seq, w1, b1, w2, b2, w3, b3):
    import ml_dtypes

    w_seq64 = np.asarray(w_seq, np.float64)
    b_seq64 = np.asarray(b_seq, np.float64)
    w164 = np.asarray(w1, np.float64)

    Weff = np.zeros((D_SEQ, D_IN, 4))
    beff = np.full(D_SEQ, float(np.asarray(b1).reshape(-1)[0]))
    for cc in (0, 1):
        for k in range(3):
            dlo, dhi = max(0, 1 - k), min(D_SEQ, D_SEQ + 1 - k)
            for kk in range(3):
                tau = cc + kk
                Weff[dlo:dhi, :, tau] += (
                    w164[0, cc, k] * w_seq64[dlo + k - 1:dhi + k - 1, :, kk]
                )
    for k in range(3):
        dlo, dhi = max(0, 1 - k), min(D_SEQ, D_SEQ + 1 - k)
        beff[dlo:dhi] += (w164[0, 0, k] + w164[0, 1, k]) * b_seq64[dlo + k - 1:dhi + k - 1]

    # bf16 bundle: wefft (2*4*128) | bandsA (2*126) | band4 (2*8)
    wb16 = np.zeros((128, WB16_COLS), np.float64)
    for t0 in (0, 1):
        for q in range(4):
            c = t0 * 512 + q * 128
            wb16[0:64, c:c + 128] = Weff[128 * q:128 * (q + 1), :, t0].T
            wb16[64:128, c:c + 128] = Weff[128 * q:128 * (q + 1), :, t0 + 2].T
    for cp in (0, 1):
        for m in range(126):
            for k in range(3):
                if m + k < 128:
                    wb16[m + k, 1024 + cp * 126 + m] = w164[0, cp, k]
        for m in range(8):
            for k in range(3):
                if m + k < 10:
                    wb16[m + k, 1276 + cp * 8 + m] = w164[0, cp, k]

    # w2r (4*128) | w2x (5*128) | w3m (64) appended to the bf16 bundle
    w2m = np.asarray(w2, np.float64)[:, :, 1].T  # (512, 128)
    w3m = np.asarray(w3, np.float64)[:, :, 1].T  # (128, 64)
    for q in range(4):
        wb16[:, 1292 + q * 128:1292 + (q + 1) * 128] = w2m[128 * q:128 * (q + 1), :]
    r0 = 0
    for j, (_, M) in enumerate(WIN):
        wb16[0:M, 1804 + j * 128:1804 + j * 128 + 128] = w2m[r0:r0 + M, :]
        r0 += M
    wb16[:, 2444:2508] = w3m
    # fp32 bundle: beff (4) | b2 | b3 | b1
    wf32 = np.zeros((128, WF32_COLS), np.float64)
    for q in range(4):
        wf32[:, q] = beff[128 * q:128 * (q + 1)]
    wf32[:, 4] = np.asarray(b2, np.float64)
    wf32[0:64, 5] = np.asarray(b3, np.float64)
    wf32[:, 6] = float(np.asarray(b1).reshape(-1)[0])

    return (
        np.ascontiguousarray(wb16, ml_dtypes.bfloat16),
        np.ascontiguousarray(wf32, np.float32),
    )


def _host_prep_core(c, refer_bf, x):
    import ml_dtypes

    bf = ml_dtypes.bfloat16
    refer_sl = np.zeros((D_IN, 4100), bf)
    lo, hi = 4096 * c - 1, 4096 * c + 4099
    glo, ghi = max(lo, 0), min(hi, T_REF)
    refer_sl[:, glo - lo:ghi - lo] = refer_bf[0, :, glo:ghi]

    xsl = x[0, 4096 * c:4096 * (c + 1), :]  # (4096, 512) fp32
    xTp = np.zeros((D_SEQ + 2, 4096), bf)
    xTp[1:-1, :] = xsl.T.astype(bf)
    xw = np.zeros((4, 128, 4096), bf)
    for j in range(4):
        xw[j] = xTp[126 * j:126 * j + 128, :]
    # chunk-major: xwc[b, p, j, c] = window_j[p, 1024b + c]
    xwc = np.ascontiguousarray(
        xw.reshape(4, 128, 4, 1024).transpose(2, 1, 0, 3)
    )
    xw4 = np.ascontiguousarray(xTp[504:514, :])
    return refer_sl, xwc, xw4


def kernel(refer, x, w_seq, b_seq, w1, b1, w2, b2, w3, b3):
    import ml_dtypes

    from concourse.bass_utils import run_bass_kernel_spmd

    refer = np.ascontiguousarray(np.asarray(refer), dtype=np.float32)
    x = np.ascontiguousarray(np.asarray(x), dtype=np.float32)
    refer_bf = refer.astype(ml_dtypes.bfloat16)

    if "nc" not in _CACHE:
        _CACHE["nc"] = _build_nc()
    nc = _CACHE["nc"]

    wb16, wf32 = _host_prep_weights(w_seq, b_seq, w1, b1, w2, b2, w3, b3)
    in_maps = []
    for c in range(N_CORES):
        refer_sl, xwc, xw4 = _host_prep_core(c, refer_bf, x)
        in_maps.append(dict(refer_sl=refer_sl, xwc=xwc, xw4=xw4, wb16=wb16, wf32=wf32))

    res = run_bass_kernel_spmd(nc, in_maps, core_ids=list(range(N_CORES)))

    final = np.zeros((32768, D_OUT, 1), np.float32)
    for c in range(N_CORES):
        r = res.results[c]["res"]  # (64, 4096)
        final[2048 * c:2048 * (c + 1), :, 0] = r[:, 0:2048].T
        final[16384 + 2048 * c:16384 + 2048 * (c + 1), :, 0] = r[:, 2048:4096].T
    return final



# revision 7
# speedup vs baseline: 1.3024x; 1.3024x over previous
"""Trainium2 Bass kernel for nn_DSNet (dense_cnn) — fp8 DoubleRow version.

Math (see reference): out = sigmoid(relu(relu(conv1(seq_splits)) @ W2 + b2) @ W3 + b3)
where seq = [conv1d(refer, w_seq) ; x^T] split into 32768 (2,512) splits.

Mapping (per core, 4096 splits = 2 super-pairs of (1024 ref + 1024 x) splits):
- ref half: linear_seq+concat+conv1 fold into one stride-2 4-tap conv with
  host-precomputed Weff[d, ch, tau]; computed as DoubleRow fp8 matmuls that
  contract (ch x 2 shifts) x (2 taus in the k-tile dim) = 256 per pass.
- x half: conv1 along the 512-dim becomes 4 banded DoubleRow matmuls with
  M=128 windows + tiny edge-patch matmuls; the 2 k-tiles carry the even/odd
  time rows (host deinterleaves x columns).
- mm2 contracts 512 via DoubleRow over d-block pairs (s1 stored fp8 x16 with
  block pairs side by side in the free dim); mm3 runs bf16 with w3/256 packed
  bit-wise into the f32 bias bundle; sigmoid output fp16.

Scales (powers of 2, exact): refer/x x1 (e4m3), Weff/w1/w2 x16, s1 fp8 x16,
h bf16 x256, w3' = w3/256, biases pre-scaled on host. Measured max rel err
~6.8e-3 vs the f32 reference.

All heavy matmuls are fp8e4 DoubleRow (0.5 cyc/row): ~2.1us ref conv +
~3.4us x conv + ~1.7us mm2 + ~1.7us mm3 of PE per core. Elementwise
(relu/hsb/sigmoid) is spread across Pool/DVE/ACT. DMA ~3.4MB/core fp8.
"""
import sys

import numpy as np

sys.path.insert(0, "/opt/trn_rl_repo")

D_IN, D_SEQ, D_H, D_OUT = 64, 512, 128, 64
T_REF = 32768
N_CORES = 8
NCH = 1024  # splits per chunk (super-pair = ref chunk + x chunk)

_CACHE = {}


def _build_nc():
    import concourse.bacc as bacc
    import concourse.bass as bass
    import concourse.mybir as mybir
    import concourse.tile as tile

    f32 = mybir.dt.float32
    bf16 = mybir.dt.bfloat16
    f16 = mybir.dt.float16
    f8 = mybir.dt.float8e4
    AF = mybir.ActivationFunctionType
    ALU = mybir.AluOpType
    DR = mybir.MatmulPerfMode.DoubleRow

    nc = bacc.Bacc("TRN2", target_bir_lowering=False, debug=False, num_devices=N_CORES)

    refer2_d = nc.dram_tensor("refer2", [128, 2, 2048], f8, kind="ExternalInput").ap()
    xwc_d = nc.dram_tensor("xwc", [2, 128, 4, 2, NCH], f8, kind="ExternalInput").ap()
    xrest_d = nc.dram_tensor("xrest", [2, 2, 2048], f8, kind="ExternalInput").ap()
    w8_d = nc.dram_tensor("w8", [128, 16, 128], f8, kind="ExternalInput").ap()
    wf32_d = nc.dram_tensor("wf32", [128, 40], f32, kind="ExternalInput").ap()
    res_d = nc.dram_tensor("res", [128, 2048], f16, kind="ExternalOutput").ap()

    with tile.TileContext(nc) as tc:
        with (
            tc.tile_pool(name="wp", bufs=1) as wp,
            tc.tile_pool(name="dp", bufs=2) as dp,
            tc.tile_pool(name="s1p", bufs=8) as s1p,
            tc.tile_pool(name="hp", bufs=2) as hp,
            tc.tile_pool(name="ppc", bufs=2, space=bass.MemorySpace.PSUM) as ppc,
            tc.tile_pool(name="pph", bufs=1, space=bass.MemorySpace.PSUM) as pph,
            tc.tile_pool(name="ppo", bufs=1, space=bass.MemorySpace.PSUM) as ppo,
        ):
            # --- PE / ACT warmup: dummy work ramps the PE pstate and loads
            # the Relu/Sigmoid ACT tables while the first DMAs are in flight.
            warm = wp.tile([1, 512], f8)
            nc.gpsimd.memset(warm[:], 0.0)
            wact = wp.tile([1, 16], f32)
            nc.scalar.activation(wact[:], warm[0:1, 0:16], AF.Relu)
            nc.scalar.activation(wact[:], warm[0:1, 0:16], AF.Sigmoid)
            psw = ppo.tile([1, 512], f32, tag="o", name="psw")
            for _ in range(7):
                nc.tensor.matmul(
                    psw[0:1, :], warm[0:1, 0:1], warm[0:1, :], start=True, stop=True
                )

            # --- weight + data loads
            w8 = wp.tile([128, 16, 128], f8)
            nc.sync.dma_start(w8[:, 0:12, :], w8_d[:, 0:12, :])
            refer2 = wp.tile([128, 2, 2048], f8)
            nc.sync.dma_start(refer2[:], refer2_d[:])
            wf32 = wp.tile([128, 40], f32)
            nc.gpsimd.dma_start(wf32[:], wf32_d[:])
            xrest = wp.tile([2, 2, 2048], f8)
            nc.gpsimd.dma_start(xrest[:], xrest_d[:])

            xt_tiles = {}

            def load_xt(b):
                xt = dp.tile([128, 4, 2, NCH], f8, tag="xt", name=f"xt_{b}")
                nc.sync.dma_start(xt[:], xwc_d[b])
                xt_tiles[b] = xt

            load_xt(0)
            nc.sync.dma_start(w8[:, 12:16, :], w8_d[:, 12:16, :])
            load_xt(1)

            res = wp.tile([128, 2048], f16)
            wb = wf32[:, 8:40].bitcast(bf16)  # [128, 64] w3/256

            # psum->sbuf relu ops: only ACT and DVE can read PSUM
            def relu_op(eng, out, psum, bias):
                if eng == "a":
                    nc.scalar.activation(out, psum, AF.Relu, bias=bias)
                else:
                    nc.vector.tensor_scalar(out, psum, bias, 0.0, ALU.add, ALU.max)

            def super_pair(s):
                xt = xt_tiles[s]
                # ---- ref conv: 4 q-blocks, DoubleRow contracting (ch,shift)x(2 taus)
                s1r = [
                    s1p.tile([128, 2, NCH], f8, tag="s1", name=f"s1r{g}_{s}")
                    for g in range(2)
                ]
                # engine schedule: balance ACT (faster/op) vs DVE
                ref_engs = ["a", "v", "a", "v"]
                x_engs = ["a", "v", "a", "v"]
                for q in range(4):
                    ps = ppc.tile([128, NCH], f32, tag="cv", name=f"psr_{s}_{q}")
                    for nh in range(4):
                        nc.tensor.matmul(
                            ps[:, 256 * nh:256 * (nh + 1)],
                            w8[:, 2 * q:2 * q + 2, :],
                            refer2[:, :, NCH * s + 256 * nh:NCH * s + 256 * (nh + 1)],
                            start=True, stop=True, perf_mode=DR,
                        )
                    relu_op(ref_engs[q], s1r[q // 2][:, q % 2, :], ps[:],
                            wf32[:, q:q + 1])
                # ---- x conv: 4 M=128 banded windows + edge patches
                s1x = [
                    s1p.tile([128, 2, NCH], f8, tag="s1", name=f"s1x{g}_{s}")
                    for g in range(2)
                ]
                for j in range(4):
                    ps = ppc.tile([128, NCH], f32, tag="cv", name=f"psx_{s}_{j}")
                    for nh in range(4):
                        o = 256 * nh
                        nc.tensor.matmul(
                            ps[:, o:o + 256],
                            w8[:, 8:10, :],
                            xt[:, j, :, o:o + 256],
                            start=True, stop=False, perf_mode=DR,
                        )
                        if j < 3:
                            prhs = xt[0:2, j + 1, :, o:o + 256]
                        else:
                            prhs = xrest[0:2, :, NCH * s + o:NCH * s + o + 256]
                        nc.tensor.matmul(
                            ps[:, o:o + 256],
                            w8[0:2, 10:12, :],
                            prhs,
                            start=False, stop=True, perf_mode=DR,
                        )
                    relu_op(x_engs[j], s1x[j // 2][:, j % 2, :], ps[:],
                            wf32[:, 6:7])
                # ---- mm2 (contract 512 over d-block pairs) + hsb, ref then x
                hcat = hp.tile([128, 2, NCH], bf16, tag="h", name=f"hcat_{s}")
                for half, s1t in ((0, s1r), (1, s1x)):
                    psh = pph.tile([128, NCH], f32, tag="hps", name=f"psh_{s}_{half}")
                    for nh in range(4):
                        o = 256 * nh
                        for g in range(2):
                            nc.tensor.matmul(
                                psh[:, o:o + 256],
                                w8[:, 12 + 2 * g:14 + 2 * g, :],
                                s1t[g][:, :, o:o + 256],
                                start=(g == 0), stop=(g == 1), perf_mode=DR,
                            )
                    if half == 0:
                        nc.scalar.activation(
                            hcat[:, half, :], psh[:], AF.Relu, bias=wf32[:, 5:6]
                        )
                    else:
                        nc.vector.tensor_scalar(
                            hcat[:, half, :], psh[:], wf32[:, 5:6], 0.0,
                            ALU.add, ALU.max,
                        )
                # ---- mm3 (bf16) + sigmoid
                pso = ppo.tile([128, NCH], f32, tag="o", name=f"pso_{s}")
                for half in range(2):
                    for n2 in range(2):
                        nc.tensor.matmul(
                            pso[64 * half:64 * half + 64, 512 * n2:512 * (n2 + 1)],
                            wb,
                            hcat[:, half, 512 * n2:512 * (n2 + 1)],
                            start=True, stop=True,
                            tile_position=(0, 64 * half),
                        )
                nc.scalar.activation(
                    res[:, NCH * s:NCH * (s + 1)], pso[:], AF.Sigmoid,
                    bias=wf32[:, 4:5],
                )
                nc.sync.dma_start(
                    res_d[:, NCH * s:NCH * (s + 1)], res[:, NCH * s:NCH * (s + 1)]
                )

            for s in range(2):
                super_pair(s)

    nc.compile()
    return nc


def _host_prep_weights(w_seq, b_seq, w1, b1, w2, b2, w3, b3):
    import ml_dtypes

    E4 = ml_dtypes.float8_e4m3
    BF = ml_dtypes.bfloat16

    w_seq64 = np.asarray(w_seq, np.float64)
    b_seq64 = np.asarray(b_seq, np.float64)
    w164 = np.asarray(w1, np.float64)

    Weff = np.zeros((D_SEQ, D_IN, 4))
    beff = np.full(D_SEQ, float(np.asarray(b1).reshape(-1)[0]))
    for cc in (0, 1):
        for k in range(3):
            dlo, dhi = max(0, 1 - k), min(D_SEQ, D_SEQ + 1 - k)
            for kk in range(3):
                tau = cc + kk
                Weff[dlo:dhi, :, tau] += (
                    w164[0, cc, k] * w_seq64[dlo + k - 1:dhi + k - 1, :, kk]
                )
    for k in range(3):
        dlo, dhi = max(0, 1 - k), min(D_SEQ, D_SEQ + 1 - k)
        beff[dlo:dhi] += (w164[0, 0, k] + w164[0, 1, k]) * b_seq64[dlo + k - 1:dhi + k - 1]

    w8 = np.zeros((128, 16, 128), np.float64)
    # wefft groups 2q+i: w8[64*bb+ch, 2q+i, m] = 16*Weff[128q+m, ch, 2bb+i]
    for q in range(4):
        for bb in range(2):
            for i in range(2):
                w8[64 * bb:64 * bb + 64, 2 * q + i, :] = (
                    16.0 * Weff[128 * q:128 * (q + 1), :, 2 * bb + i].T
                )
    # band groups 8+i: w8[p, 8+i, m] = 16*w1[0,i,p-m], p-m in {0,1,2}
    for i in range(2):
        for m in range(128):
            for k in range(3):
                if m + k < 128:
                    w8[m + k, 8 + i, m] = 16.0 * w164[0, i, k]
        # patch groups 10+i
        w8[0, 10 + i, 126] = 16.0 * w164[0, i, 2]
        w8[0, 10 + i, 127] = 16.0 * w164[0, i, 1]
        w8[1, 10 + i, 127] = 16.0 * w164[0, i, 2]
    # w2 groups 12+2g+i: w8[p, 12+2g+i, e] = 16*w2[e, 128*(2g+i)+p]
    w2m = np.asarray(w2, np.float64)[:, :, 1]  # (128, 512)
    for g in range(2):
        for i in range(2):
            blk = 2 * g + i
            w8[:, 12 + 2 * g + i, :] = 16.0 * w2m[:, 128 * blk:128 * (blk + 1)].T

    wf32 = np.zeros((128, 40), np.float32)
    for q in range(4):
        wf32[:, q] = 16.0 * beff[128 * q:128 * (q + 1)]
    b3a = np.asarray(b3, np.float64)
    wf32[:, 4] = np.tile(b3a, 2).astype(np.float32)
    wf32[:, 5] = 256.0 * np.asarray(b2, np.float64)
    wf32[:, 6] = 16.0 * float(np.asarray(b1).reshape(-1)[0])
    # w3' = w3/256 as bf16, bit-packed into f32 columns 8..40
    w3p = np.ascontiguousarray((np.asarray(w3, np.float64)[:, :, 1].T / 256.0)).astype(BF)
    wf32[:, 8:40] = w3p.view(np.float32)

    return np.ascontiguousarray(w8.astype(E4)), np.ascontiguousarray(wf32)


def _host_prep_data(refer, x):
    """Global fp8 conversion + padded transposes shared by all cores."""
    import ml_dtypes

    E4 = ml_dtypes.float8_e4m3
    refer8p = np.zeros((D_IN, T_REF + 4), E4)
    refer8p[:, 1:T_REF + 1] = np.asarray(refer[0], np.float32).astype(E4)
    x8 = np.asarray(x[0], np.float32).astype(E4)  # (T, 512)
    xTpad = np.zeros((513, T_REF), E4)
    xTpad[1:513] = x8.T
    return refer8p, xTpad


def _host_prep_core(c, refer8p, xTpad):
    import ml_dtypes

    E4 = ml_dtypes.float8_e4m3
    # refer2[p, i, cn] = refer[ch, 4096c + 2cn + i - 1 + 2*(p>=64)]
    # refer8p col t -> index t+1
    refer2 = np.zeros((128, 2, 2048), E4)
    base = 4096 * c
    for i in range(2):
        refer2[0:64, i, :] = refer8p[:, base + i:base + i + 4096:2]
        refer2[64:128, i, :] = refer8p[:, base + i + 2:base + i + 4096 + 2:2]
    # xwc[b, p, j, i, n] = xTpad[128j + p, 4096c + 2048b + 2n + i]
    xwc = np.zeros((2, 128, 4, 2, NCH), E4)
    for b in range(2):
        t0 = 4096 * c + 2048 * b
        for j in range(4):
            blk = xTpad[128 * j:128 * j + 128, t0:t0 + 2048]
            xwc[b, :, j, 0, :] = blk[:, 0::2]
            xwc[b, :, j, 1, :] = blk[:, 1::2]
    xrest = np.zeros((2, 2, 2048), E4)
    xrest[0, 0, :] = xTpad[512, 4096 * c:4096 * (c + 1):2]
    xrest[0, 1, :] = xTpad[512, 4096 * c + 1:4096 * (c + 1):2]
    return refer2, np.ascontiguousarray(xwc), xrest


def kernel(refer, x, w_seq, b_seq, w1, b1, w2, b2, w3, b3):
    from concourse.bass_utils import run_bass_kernel_spmd

    if "nc" not in _CACHE:
        _CACHE["nc"] = _build_nc()
    nc = _CACHE["nc"]

    w8, wf32 = _host_prep_weights(w_seq, b_seq, w1, b1, w2, b2, w3, b3)
    refer8p, xTpad = _host_prep_data(refer, x)
    in_maps = []
    for c in range(N_CORES):
        refer2, xwc, xrest = _host_prep_core(c, refer8p, xTpad)
        in_maps.append(
            dict(refer2=refer2, xwc=xwc, xrest=xrest, w8=w8, wf32=wf32)
        )

    res = run_bass_kernel_spmd(nc, in_maps, core_ids=list(range(N_CORES)))

    final = np.zeros((32768, D_OUT, 1), np.float32)
    for c in range(N_CORES):
        r = np.asarray(res.results[c]["res"], np.float32)  # (128, 2048)
        final[2048 * c:2048 * (c + 1), :, 0] = r[0:64, :].T
        final[16384 + 2048 * c:16384 + 2048 * (c + 1), :, 0] = r[64:128, :].T
    return final


# revision 8
# speedup vs baseline: 1.4714x; 1.1298x over previous
"""Trainium2 Bass kernel for nn_DSNet (dense_cnn) — fp8 DoubleRow version.

Math (see reference): out = sigmoid(relu(relu(conv1(seq_splits)) @ W2 + b2) @ W3 + b3)
where seq = [conv1d(refer, w_seq) ; x^T] split into 32768 (2,512) splits.

Mapping (per core, 4096 splits = 2 super-pairs of (1024 ref + 1024 x) splits):
- ref half: linear_seq+concat+conv1 fold into one stride-2 4-tap conv with
  host-precomputed Weff[d, ch, tau]; computed as DoubleRow fp8 matmuls that
  contract (ch x 2 shifts) x (2 taus in the k-tile dim) = 256 per pass.
- x half: conv1 along the 512-dim becomes 4 banded DoubleRow matmuls with
  M=128 windows + tiny edge-patch matmuls; the 2 k-tiles carry the even/odd
  time rows (host deinterleaves x columns).
- mm2 contracts 512 via DoubleRow over d-block pairs (s1 stored fp8 x16 with
  block pairs side by side in the free dim); mm3 runs bf16 with w3/256 packed
  bit-wise into the f32 bias bundle; sigmoid output fp16.

Scales (powers of 2, exact): refer/x x1 (e4m3), Weff/w1/w2 x16, s1 fp8 x16,
h bf16 x256, w3' = w3/256, biases pre-scaled on host. Measured max rel err
~6.8e-3 vs the f32 reference.

All heavy matmuls are fp8e4 DoubleRow (0.5 cyc/row): ~2.1us ref conv +
~3.4us x conv + ~1.7us mm2 + ~1.7us mm3 of PE per core. Elementwise
(relu/hsb/sigmoid) is spread across Pool/DVE/ACT. DMA ~3.4MB/core fp8.
"""
import sys

import numpy as np

sys.path.insert(0, "/opt/trn_rl_repo")

D_IN, D_SEQ, D_H, D_OUT = 64, 512, 128, 64
T_REF = 32768
N_CORES = 8
NCH = 1024  # splits per chunk (super-pair = ref chunk + x chunk)

_CACHE = {}


def _build_nc():
    import concourse.bacc as bacc
    import concourse.bass as bass
    import concourse.mybir as mybir
    import concourse.tile as tile

    f32 = mybir.dt.float32
    bf16 = mybir.dt.bfloat16
    f16 = mybir.dt.float16
    f8 = mybir.dt.float8e4
    AF = mybir.ActivationFunctionType
    ALU = mybir.AluOpType
    DR = mybir.MatmulPerfMode.DoubleRow

    nc = bacc.Bacc("TRN2", target_bir_lowering=False, debug=False, num_devices=N_CORES)

    refer2_d = nc.dram_tensor("refer2", [128, 2, 2048], f8, kind="ExternalInput").ap()
    xwc_d = nc.dram_tensor("xwc", [2, 128, 4, 2, NCH], f8, kind="ExternalInput").ap()
    xrest_d = nc.dram_tensor("xrest", [2, 2, 2048], f8, kind="ExternalInput").ap()
    w8_d = nc.dram_tensor("w8", [128, 16, 128], f8, kind="ExternalInput").ap()
    wf32_d = nc.dram_tensor("wf32", [128, 40], f32, kind="ExternalInput").ap()
    res_d = nc.dram_tensor("res", [128, 2048], f16, kind="ExternalOutput").ap()

    with tile.TileContext(nc) as tc:
        with (
            tc.tile_pool(name="wp", bufs=1) as wp,
            tc.tile_pool(name="dp", bufs=2) as dp,
            tc.tile_pool(name="s1p", bufs=8) as s1p,
            tc.tile_pool(name="hp", bufs=2) as hp,
            tc.tile_pool(name="pp", bufs=4, space=bass.MemorySpace.PSUM) as pp,
        ):
            # --- PE / ACT warmup: dummy work ramps the PE pstate and loads
            # the Relu/Sigmoid ACT tables while the first DMAs are in flight.
            warm = wp.tile([1, 512], f8)
            nc.gpsimd.memset(warm[:], 0.0)
            wact = wp.tile([1, 16], f32)
            nc.scalar.activation(wact[:], warm[0:1, 0:16], AF.Relu)
            nc.scalar.activation(wact[:], warm[0:1, 0:16], AF.Sigmoid)
            psw = pp.tile([1, NCH], f32, tag="ps", name="psw")
            for _ in range(8):
                nc.tensor.matmul(
                    psw[0:1, 0:512], warm[0:1, 0:1], warm[0:1, :], start=True,
                    stop=True,
                )

            # --- weight + data loads (SP/HWDGE for the big ones, SWDGE for
            # the small fry). Order: first-needed first.
            w8 = wp.tile([128, 16, 128], f8)
            nc.sync.dma_start(w8[:, 0:12, :], w8_d[:, 0:12, :])
            refer2 = wp.tile([128, 2, 2048], f8)
            nc.sync.dma_start(refer2[:, :, 0:1024], refer2_d[:, :, 0:1024])
            wf32 = wp.tile([128, 40], f32)
            nc.gpsimd.dma_start(wf32[:], wf32_d[:])
            xrest = wp.tile([2, 2, 2048], f8)
            nc.gpsimd.dma_start(xrest[:], xrest_d[:])

            xt_tiles = {}

            def load_xt(b):
                xt = dp.tile([128, 4, 2, NCH], f8, tag="xt", name=f"xt_{b}")
                nc.sync.dma_start(xt[:], xwc_d[b])
                xt_tiles[b] = xt

            load_xt(0)
            nc.sync.dma_start(refer2[:, :, 1024:2048], refer2_d[:, :, 1024:2048])
            nc.sync.dma_start(w8[:, 12:16, :], w8_d[:, 12:16, :])
            load_xt(1)

            res = wp.tile([128, 2048], f16)
            wb = wf32[:, 8:40].bitcast(bf16)  # [128, 64] w3/256

            # psum->sbuf relu ops: only ACT and DVE can read PSUM
            def relu_op(eng, out, psum, bias):
                if eng == "a":
                    nc.scalar.activation(out, psum, AF.Relu, bias=bias)
                else:
                    nc.vector.tensor_scalar(out, psum, bias, 0.0, ALU.add, ALU.max)

            s1r_tiles, s1x_tiles = {}, {}

            def conv_ref(s):
                s1r = [
                    s1p.tile([128, 2, NCH], f8, tag="s1", name=f"s1r{g}_{s}")
                    for g in range(2)
                ]
                for q in range(4):
                    ps = pp.tile([128, NCH], f32, tag="ps", name=f"psr_{s}_{q}")
                    for nh in range(4):
                        nc.tensor.matmul(
                            ps[:, 256 * nh:256 * (nh + 1)],
                            w8[:, 2 * q:2 * q + 2, :],
                            refer2[:, :, NCH * s + 256 * nh:NCH * s + 256 * (nh + 1)],
                            start=True, stop=True, perf_mode=DR,
                        )
                    relu_op("av"[q % 2], s1r[q // 2][:, q % 2, :], ps[:],
                            wf32[:, q:q + 1])
                s1r_tiles[s] = s1r

            def conv_x(s):
                xt = xt_tiles[s]
                s1x = [
                    s1p.tile([128, 2, NCH], f8, tag="s1", name=f"s1x{g}_{s}")
                    for g in range(2)
                ]
                for j in range(4):
                    ps = pp.tile([128, NCH], f32, tag="ps", name=f"psx_{s}_{j}")
                    for nh in range(4):
                        o = 256 * nh
                        nc.tensor.matmul(
                            ps[:, o:o + 256],
                            w8[:, 8:10, :],
                            xt[:, j, :, o:o + 256],
                            start=True, stop=False, perf_mode=DR,
                        )
                        if j < 3:
                            prhs = xt[0:2, j + 1, :, o:o + 256]
                        else:
                            prhs = xrest[0:2, :, NCH * s + o:NCH * s + o + 256]
                        nc.tensor.matmul(
                            ps[:, o:o + 256],
                            w8[0:2, 10:12, :],
                            prhs,
                            start=False, stop=True, perf_mode=DR,
                        )
                    relu_op("av"[j % 2], s1x[j // 2][:, j % 2, :], ps[:],
                            wf32[:, 6:7])
                s1x_tiles[s] = s1x

            def tail(s, last=False):
                hcat = hp.tile([128, 2, NCH], bf16, tag="h", name=f"hcat_{s}")
                psh = {}
                for half, s1t in ((0, s1r_tiles[s]), (1, s1x_tiles[s])):
                    ph = pp.tile([128, NCH], f32, tag="ps", name=f"psh_{s}_{half}")
                    psh[half] = ph
                    for nh in range(4):
                        o = 256 * nh
                        for g in range(2):
                            nc.tensor.matmul(
                                ph[:, o:o + 256],
                                w8[:, 12 + 2 * g:14 + 2 * g, :],
                                s1t[g][:, :, o:o + 256],
                                start=(g == 0), stop=(g == 1), perf_mode=DR,
                            )
                for half in range(2):
                    relu_op("av"[half], hcat[:, half, :], psh[half][:],
                            wf32[:, 5:6])
                pso = pp.tile([128, NCH], f32, tag="ps", name=f"pso_{s}")
                for half in range(2):
                    for n2 in range(2):
                        nc.tensor.matmul(
                            pso[64 * half:64 * half + 64, 512 * n2:512 * (n2 + 1)],
                            wb,
                            hcat[:, half, 512 * n2:512 * (n2 + 1)],
                            start=True, stop=True,
                            tile_position=(0, 64 * half),
                        )
                if not last:
                    nc.scalar.activation(
                        res[:, NCH * s:NCH * (s + 1)], pso[:], AF.Sigmoid,
                        bias=wf32[:, 4:5],
                    )
                    nc.sync.dma_start(
                        res_d[:, NCH * s:NCH * (s + 1)],
                        res[:, NCH * s:NCH * (s + 1)],
                    )
                else:
                    # split the final sigmoid + store so the tail pipelines
                    for n2 in range(2):
                        o = NCH * s + 512 * n2
                        nc.scalar.activation(
                            res[:, o:o + 512], pso[:, 512 * n2:512 * (n2 + 1)],
                            AF.Sigmoid, bias=wf32[:, 4:5],
                        )
                        nc.sync.dma_start(res_d[:, o:o + 512], res[:, o:o + 512])

            conv_ref(0)
            conv_x(0)
            conv_ref(1)
            tail(0)
            conv_x(1)
            tail(1, last=True)

    nc.compile()
    return nc


def _host_prep_weights(w_seq, b_seq, w1, b1, w2, b2, w3, b3):
    import ml_dtypes

    E4 = ml_dtypes.float8_e4m3
    BF = ml_dtypes.bfloat16

    w_seq64 = np.asarray(w_seq, np.float64)
    b_seq64 = np.asarray(b_seq, np.float64)
    w164 = np.asarray(w1, np.float64)

    Weff = np.zeros((D_SEQ, D_IN, 4))
    beff = np.full(D_SEQ, float(np.asarray(b1).reshape(-1)[0]))
    for cc in (0, 1):
        for k in range(3):
            dlo, dhi = max(0, 1 - k), min(D_SEQ, D_SEQ + 1 - k)
            for kk in range(3):
                tau = cc + kk
                Weff[dlo:dhi, :, tau] += (
                    w164[0, cc, k] * w_seq64[dlo + k - 1:dhi + k - 1, :, kk]
                )
    for k in range(3):
        dlo, dhi = max(0, 1 - k), min(D_SEQ, D_SEQ + 1 - k)
        beff[dlo:dhi] += (w164[0, 0, k] + w164[0, 1, k]) * b_seq64[dlo + k - 1:dhi + k - 1]

    w8 = np.zeros((128, 16, 128), np.float64)
    # wefft groups 2q+i: w8[64*bb+ch, 2q+i, m] = 16*Weff[128q+m, ch, 2bb+i]
    for q in range(4):
        for bb in range(2):
            for i in range(2):
                w8[64 * bb:64 * bb + 64, 2 * q + i, :] = (
                    16.0 * Weff[128 * q:128 * (q + 1), :, 2 * bb + i].T
                )
    # band groups 8+i: w8[p, 8+i, m] = 16*w1[0,i,p-m], p-m in {0,1,2}
    for i in range(2):
        for m in range(128):
            for k in range(3):
                if m + k < 128:
                    w8[m + k, 8 + i, m] = 16.0 * w164[0, i, k]
        # patch groups 10+i
        w8[0, 10 + i, 126] = 16.0 * w164[0, i, 2]
        w8[0, 10 + i, 127] = 16.0 * w164[0, i, 1]
        w8[1, 10 + i, 127] = 16.0 * w164[0, i, 2]
    # w2 groups 12+2g+i: w8[p, 12+2g+i, e] = 16*w2[e, 128*(2g+i)+p]
    w2m = np.asarray(w2, np.float64)[:, :, 1]  # (128, 512)
    for g in range(2):
        for i in range(2):
            blk = 2 * g + i
            w8[:, 12 + 2 * g + i, :] = 16.0 * w2m[:, 128 * blk:128 * (blk + 1)].T

    wf32 = np.zeros((128, 40), np.float32)
    for q in range(4):
        wf32[:, q] = 16.0 * beff[128 * q:128 * (q + 1)]
    b3a = np.asarray(b3, np.float64)
    wf32[:, 4] = np.tile(b3a, 2).astype(np.float32)
    wf32[:, 5] = 256.0 * np.asarray(b2, np.float64)
    wf32[:, 6] = 16.0 * float(np.asarray(b1).reshape(-1)[0])
    # w3' = w3/256 as bf16, bit-packed into f32 columns 8..40
    w3p = np.ascontiguousarray((np.asarray(w3, np.float64)[:, :, 1].T / 256.0)).astype(BF)
    wf32[:, 8:40] = w3p.view(np.float32)

    return np.ascontiguousarray(w8.astype(E4)), np.ascontiguousarray(wf32)


def _host_prep_data(refer, x):
    """Global fp8 conversion + padded transposes shared by all cores."""
    import ml_dtypes

    E4 = ml_dtypes.float8_e4m3
    refer8p = np.zeros((D_IN, T_REF + 4), E4)
    refer8p[:, 1:T_REF + 1] = np.asarray(refer[0], np.float32).astype(E4)
    x8 = np.asarray(x[0], np.float32).astype(E4)  # (T, 512)
    xTpad = np.zeros((513, T_REF), E4)
    xTpad[1:513] = x8.T
    return refer8p, xTpad


def _host_prep_core(c, refer8p, xTpad):
    import ml_dtypes

    E4 = ml_dtypes.float8_e4m3
    # refer2[p, i, cn] = refer[ch, 4096c + 2cn + i - 1 + 2*(p>=64)]
    # refer8p col t -> index t+1
    refer2 = np.zeros((128, 2, 2048), E4)
    base = 4096 * c
    for i in range(2):
        refer2[0:64, i, :] = refer8p[:, base + i:base + i + 4096:2]
        refer2[64:128, i, :] = refer8p[:, base + i + 2:base + i + 4096 + 2:2]
    # xwc[b, p, j, i, n] = xTpad[128j + p, 4096c + 2048b + 2n + i]
    xwc = np.zeros((2, 128, 4, 2, NCH), E4)
    for b in range(2):
        t0 = 4096 * c + 2048 * b
        for j in range(4):
            blk = xTpad[128 * j:128 * j + 128, t0:t0 + 2048]
            xwc[b, :, j, 0, :] = blk[:, 0::2]
            xwc[b, :, j, 1, :] = blk[:, 1::2]
    xrest = np.zeros((2, 2, 2048), E4)
    xrest[0, 0, :] = xTpad[512, 4096 * c:4096 * (c + 1):2]
    xrest[0, 1, :] = xTpad[512, 4096 * c + 1:4096 * (c + 1):2]
    return refer2, np.ascontiguousarray(xwc), xrest


def kernel(refer, x, w_seq, b_seq, w1, b1, w2, b2, w3, b3):
    from concourse.bass_utils import run_bass_kernel_spmd

    if "nc" not in _CACHE:
        _CACHE["nc"] = _build_nc()
    nc = _CACHE["nc"]

    w8, wf32 = _host_prep_weights(w_seq, b_seq, w1, b1, w2, b2, w3, b3)
    refer8p, xTpad = _host_prep_data(refer, x)
    in_maps = []
    for c in range(N_CORES):
        refer2, xwc, xrest = _host_prep_core(c, refer8p, xTpad)
        in_maps.append(
            dict(refer2=refer2, xwc=xwc, xrest=xrest, w8=w8, wf32=wf32)
        )

    res = run_bass_kernel_spmd(nc, in_maps, core_ids=list(range(N_CORES)))

    final = np.zeros((32768, D_OUT, 1), np.float32)
    for c in range(N_CORES):
        r = np.asarray(res.results[c]["res"], np.float32)  # (128, 2048)
        final[2048 * c:2048 * (c + 1), :, 0] = r[0:64, :].T
        final[16384 + 2048 * c:16384 + 2048 * (c + 1), :, 0] = r[64:128, :].T
    return final
